# revision 1
# baseline (speedup 1.0000x reference)
"""Trainium2 Bass kernel for nn_Discriminator_80195629351349.

Pairwise-column MLP discriminator over k-space columns.

Math (matching the jax reference):
  F[b, w, ch] = |kspace[b, c, h, w]|  (ch = c*H + h)
  Pq = Fq @ W1[:, :CH].T ;  Pa = Fa @ W1[:, CH:].T          [B, W, 18]
  out[b, wi, wc] = sigmoid(W4 @ r3 + b4),  r3 = relu-chain of
                   relu(Pq[wi] + Pa[wc] + b1) through W2, W3
  heat[b, wi] = sum_wc out[b, wi, wc] * cmask[b, wc] / denom[b]
  result[b, h, w] = heat[b, w] if acquiring_mask[b, w] > 0 else 0

Only columns wi with acquiring_mask>0 (16 of 384) contribute to the
output, and the wc sum runs only over [left, right) (191 of 384
columns), so the kernel computes exactly that slice.

Sharding: 8 cores = (batch b in 0..3) x (wc half s in 0..1). Each core
loads its own slice of acquired/acquiring k-space columns (host-side
slicing = the sharding step), computes the column features + all pair
MLP evaluations on-device, and returns partial heat sums [4, NL]
(4 wi-quadrants x NL wi-slots-per-quadrant). Host combines.

On-device layout trick: the 18-channel MLP is packed 4x block-diagonal
across the 128 partitions (quadrant j = partitions 32j..32j+17), so
layers 2-4 are single matmuls with N = NL*NWC <= 512 free columns.
"""

import math
import os

import numpy as np

B, C, H, W = 4, 8, 384, 384
CH = C * H            # 3072 features per column
P = 128               # SBUF partitions
KT = CH // P          # 24 contraction tiles
CHANS = 18            # MLP width
NCORES = 8

# constant-block column layout (one [128, CW] DMA carries everything)
_C_W1AT = 0
_C_W1QT = _C_W1AT + KT * CHANS            # 432
_C_W2 = _C_W1QT + KT * CHANS              # 864
_C_W3 = _C_W2 + P                         # 992
_C_W4 = _C_W3 + P                         # 1120
_C_REP = _C_W4 + 4                        # 1124  quadrant-replication selector
_C_B2 = _C_REP + P                        # 1252
_C_B3 = _C_B2 + 1                         # 1253
_C_B1 = _C_B3 + 1                         # 1254
_C_B4 = _C_B1 + 1                         # 1255
_C_CM = _C_B4 + 1                         # 1256  (+NWC)

_prog_cache: dict = {}
LAST_RESULTS = None   # BassKernelResults of the most recent run (for test.py)


def _build_program(NWC: int, NL: int):
    """Build the SPMD Bass/Tile program for one core.

    NWC: number of wc (acquired) columns this core handles.
    NL:  wi slots per partition-quadrant (total wi slots = 4*NL).
    """
    import concourse.bass as bass
    import concourse.tile as tile
    from concourse import bacc, mybir

    f32 = mybir.dt.float32
    NS = 4 * NL          # wi slots
    NF = NL * NWC        # free columns of the pair block
    CW = _C_CM + NWC
    assert NF <= 512

    nc = bacc.Bacc("TRN2", debug=False)

    # ---- DRAM I/O (per-core shapes; host fills per (b, s)) ----
    aks = nc.dram_tensor("aks", [CH, 2 * NWC], f32, kind="ExternalInput")
    qks = nc.dram_tensor("qks", [CH, 2 * NS], f32, kind="ExternalInput")
    cst = nc.dram_tensor("cst", [P, CW], f32, kind="ExternalInput")
    hp = nc.dram_tensor("hp", [4, NL], f32, kind="ExternalOutput")

    AF = mybir.ActivationFunctionType
    ALU = mybir.AluOpType

    KPC = 6                      # k-tiles per DMA chunk for acquired data
    NCHUNK = KT // KPC           # 4 chunks

    with tile.TileContext(nc) as tc:
        with (
            tc.tile_pool(name="consts", bufs=1) as consts,
            tc.tile_pool(name="adata", bufs=NCHUNK) as adata,
            tc.tile_pool(name="qdata", bufs=1) as qdata,
            tc.tile_pool(name="feat", bufs=1) as feat,
            tc.tile_pool(name="sq", bufs=3) as sqp,
            tc.tile_pool(name="m2", bufs=3) as m2p,
            tc.tile_pool(name="mlp", bufs=1) as mlp,
            tc.tile_pool(name="psA", bufs=1, space="PSUM") as psA,
            tc.tile_pool(name="psB", bufs=1, space="PSUM") as psB,
        ):
            # ---- q k-space first on the scalar DMA queue (tiny, feeds the
            # head of the DVE pipeline), then the constant block ----
            qks_r = qks[:].rearrange("(k p) n -> p k n", p=P)
            qc = qdata.tile([P, KT * 2 * NS], f32, tag="qc")
            nc.scalar.dma_start(out=qc.rearrange("p (k n) -> p k n", k=KT),
                                in_=qks_r)
            cst_s = consts.tile([P, CW], f32, tag="cst")
            nc.scalar.dma_start(out=cst_s, in_=cst[:])
            w2bd_s = cst_s[:, _C_W2:_C_W2 + P]
            w3bd_s = cst_s[:, _C_W3:_C_W3 + P]
            w4bd_s = cst_s[:, _C_W4:_C_W4 + 4]
            b2_s = cst_s[:, _C_B2:_C_B2 + 1]
            b3_s = cst_s[:, _C_B3:_C_B3 + 1]
            b1_s = cst_s[0:CHANS, _C_B1:_C_B1 + 1]
            b4_s = cst_s[0:4, _C_B4:_C_B4 + 1]
            cm_s = cst_s[0:4, _C_CM:_C_CM + NWC]

            # ---- q path first: small and cheap, frees PE to start early ----
            fq_s = feat.tile([P, KT, NS], f32, tag="fq")
            sqq = sqp.tile([P, KT * 2 * NS], f32, tag="sqq")
            nc.vector.tensor_mul(sqq, qc, qc)
            vq = sqq.rearrange("p (n r) -> p n r", r=2)
            m2q = m2p.tile([P, KT * NS], f32, tag="m2q")
            nc.vector.tensor_add(m2q, vq[:, :, 0], vq[:, :, 1])
            nc.scalar.sqrt(fq_s.rearrange("p k n -> p (k n)"), m2q)

            psum_pq = psA.tile([CHANS, NS], f32, tag="ppq")
            for k in range(KT):
                nc.tensor.matmul(
                    out=psum_pq,
                    lhsT=cst_s[:, _C_W1QT + k * CHANS:_C_W1QT + (k + 1) * CHANS],
                    rhs=fq_s[:, k, :],
                    start=(k == 0),
                    stop=(k == KT - 1),
                )
            pq_s = mlp.tile([CHANS, NS], f32, tag="pq")
            nc.vector.tensor_copy(pq_s, psum_pq)
            pq4 = mlp.tile([P, NL], f32, tag="pq4")
            nc.vector.memset(pq4, 0.0)
            for j in range(4):
                eng = nc.scalar if j % 2 == 0 else nc.sync
                eng.dma_start(out=pq4[32 * j:32 * j + CHANS, :],
                              in_=pq_s[:, j * NL:(j + 1) * NL])

            # ---- stream acquired k-space; ch = k*128 + p ----
            aks_r = aks[:].rearrange("(k p) n -> p k n", p=P)
            achunks = []
            for i in range(NCHUNK):
                ac = adata.tile([P, KPC * 2 * NWC], f32, tag=f"ac{i}")
                eng = nc.sync if i % 2 == 0 else nc.scalar
                eng.dma_start(
                    out=ac.rearrange("p (k n) -> p k n", k=KPC),
                    in_=aks_r[:, i * KPC:(i + 1) * KPC, :])
                achunks.append(ac)

            # ---- |z| = sqrt(re^2 + im^2), one fat op per chunk ----
            fa_s = feat.tile([P, KT, NWC], f32, tag="fa")
            fa_flat = fa_s.rearrange("p k n -> p (k n)")
            psum_pa = psA.tile([CHANS, NWC], f32, tag="ppa")
            for i in range(NCHUNK):
                src = achunks[i]
                sqt = sqp.tile([P, KPC * 2 * NWC], f32, tag="sqa")
                nc.vector.tensor_mul(sqt, src, src)
                v = sqt.rearrange("p (n r) -> p n r", r=2)
                m2 = m2p.tile([P, KPC * NWC], f32, tag="m2a")
                nc.vector.tensor_add(m2, v[:, :, 0], v[:, :, 1])
                nc.scalar.sqrt(
                    fa_flat[:, i * KPC * NWC:(i + 1) * KPC * NWC], m2)
                for k in range(i * KPC, (i + 1) * KPC):
                    nc.tensor.matmul(
                        out=psum_pa,
                        lhsT=cst_s[:, _C_W1AT + k * CHANS:
                                   _C_W1AT + (k + 1) * CHANS],
                        rhs=fa_s[:, k, :],
                        start=(k == 0),
                        stop=(k == KT - 1),
                    )

            # prefetch the sigmoid ACT table right after the last a-sqrt so
            # the real sigmoid doesn't pay the 1.3us table swap on the tail
            sigdummy = mlp.tile([1, 1], f32, tag="sigd")
            nc.scalar.activation(out=sigdummy, in_=fa_s[0:1, KT - 1, 0:1],
                                 func=AF.Sigmoid)

            # Pa + b1 on DVE (keeps ACT free for the sigmoid table)
            pa_s = mlp.tile([CHANS, NWC], f32, tag="pa")
            nc.vector.tensor_scalar(out=pa_s, in0=psum_pa, scalar1=b1_s,
                                    scalar2=None, op0=ALU.add)
            # replicate into the 4 partition quadrants with one matmul
            # against a constant selector (pad rows come out zero)
            pa4 = psA.tile([P, NWC], f32, tag="pa4")
            nc.tensor.matmul(out=pa4, lhsT=cst_s[0:CHANS, _C_REP:_C_REP + P],
                             rhs=pa_s, start=True, stop=True)

            # ---- pair MLP, pipelined as two wi-slot halves so PE matmuls
            # of one half overlap DVE relus of the other ----
            NLH = max(NL // 2, 1)
            halves = [(0, NLH), (NLH, NL)] if NL > 1 else [(0, 1)]
            h1p = mlp.tile([P, NF], f32, tag="h1p")
            hp_s = mlp.tile([4, NL], f32, tag="hps")
            scr = mlp.tile([4, NL, NWC], f32, tag="scr")
            for hi, (l0, l1) in enumerate(halves):
                HF = (l1 - l0) * NWC
                off = l0 * NWC
                h1h = h1p[:, off:off + HF]
                for lw in range(l0, l1):
                    nc.vector.tensor_scalar(
                        out=h1p[:, lw * NWC:(lw + 1) * NWC],
                        in0=pa4,
                        scalar1=pq4[:, lw:lw + 1],
                        scalar2=0.0,
                        op0=ALU.add,
                        op1=ALU.max,
                    )
                psum2 = psB.tile([P, HF], f32, tag=f"ps2_{hi}")
                nc.tensor.matmul(out=psum2, lhsT=w2bd_s, rhs=h1h,
                                 start=True, stop=True)
                h2p = mlp.tile([P, HF], f32, tag=f"h2p_{hi}")
                nc.vector.tensor_scalar(out=h2p, in0=psum2, scalar1=b2_s,
                                        scalar2=0.0, op0=ALU.add, op1=ALU.max)
                psum3 = psB.tile([P, HF], f32, tag=f"ps3_{hi}")
                nc.tensor.matmul(out=psum3, lhsT=w3bd_s, rhs=h2p,
                                 start=True, stop=True)
                h3p = mlp.tile([P, HF], f32, tag=f"h3p_{hi}")
                nc.vector.tensor_scalar(out=h3p, in0=psum3, scalar1=b3_s,
                                        scalar2=0.0, op0=ALU.add, op1=ALU.max)
                psum4 = psB.tile([4, HF], f32, tag="ps4")
                nc.tensor.matmul(out=psum4, lhsT=w4bd_s, rhs=h3p,
                                 start=True, stop=True)
                sig = mlp.tile([4, HF], f32, tag=f"sig_{hi}")
                nc.scalar.activation(out=sig, in_=psum4, func=AF.Sigmoid,
                                     bias=b4_s, scale=1.0)
                for lw in range(l0, l1):
                    nc.vector.tensor_mul(
                        scr[:, lw, :],
                        sig[:, (lw - l0) * NWC:(lw - l0 + 1) * NWC], cm_s)

            # ---- heat[j, lw] = sum_c sig[j, lw*NWC+c] * cm[c] ----
            nc.vector.reduce_sum(hp_s, scr, axis=mybir.AxisListType.X)
            nc.sync.dma_start(out=hp[:], in_=hp_s)

    nc.finalize()
    return nc


def _run_sim(nc, in_maps):
    """CoreSim (CPU instruction simulator) path for local dev testing."""
    from concourse.bass_interp import MultiCoreSim
    from concourse.bass_utils import BassKernelResults

    sim = MultiCoreSim(nc, num_cores=len(in_maps))
    for core_id, core in sim.cores.items():
        for name, arr in in_maps[core_id].items():
            core.tensor(name)[:] = arr
    sim.simulate()
    results = [
        {"hp": np.array(sim.cores[i].tensor("hp"))} for i in range(len(in_maps))
    ]
    return BassKernelResults(results=results, instructions_and_trace=None,
                             profile_json=None, exec_time_ns=None)


def _mask_geometry(acquired_mask, acquiring_mask):
    """Replicates the reference's left/right/cmask/denom logic exactly."""
    am = np.asarray(acquired_mask, np.float32)
    qm = np.asarray(acquiring_mask, np.float32)
    mid = W // 2
    right = mid + np.argmax(am[:, mid:] < 1.0, axis=1)
    left = np.argmax(am[:, :mid][:, ::-1] < 1.0, axis=1) + 1
    cols = np.arange(W)
    cmask = (cols[None, :] >= left[:, None]) & (cols[None, :] < right[:, None])
    denom = (right - left).astype(np.float32)
    active = [np.nonzero(qm[b] > 0)[0] for b in range(B)]
    return left.astype(int), right.astype(int), cmask, denom, active


def kernel(acquired_kspace, acquiring_kspace, acquired_mask, acquiring_mask,
           W1, b1, W2, b2, W3, b3, W4, b4):
    global LAST_RESULTS
    from concourse.bass_utils import run_bass_kernel_spmd

    acquired_kspace = np.ascontiguousarray(np.asarray(acquired_kspace, np.float32))
    acquiring_kspace = np.ascontiguousarray(np.asarray(acquiring_kspace, np.float32))
    W1 = np.asarray(W1, np.float32)
    b1 = np.asarray(b1, np.float32)
    W2 = np.asarray(W2, np.float32)
    b2 = np.asarray(b2, np.float32)
    W3 = np.asarray(W3, np.float32)
    b3 = np.asarray(b3, np.float32)
    W4 = np.asarray(W4, np.float32)
    b4 = np.asarray(b4, np.float32)

    left, right, cmask, denom, active = _mask_geometry(acquired_mask, acquiring_mask)

    nmax = max(len(a) for a in active)
    out = np.zeros((B, H, W), np.float32)
    if nmax == 0:
        return out

    span = max(int((right - left).max()), 1)
    NL = max(1, math.ceil(nmax / 4))          # wi slots per quadrant
    NWC = max(1, math.ceil(span / 2))         # wc columns per core
    NS = 4 * NL
    assert NL * NWC <= 512, (NL, NWC)

    # ---- shared constant block [128, CW] ----
    CW = _C_CM + NWC
    W1q, W1a = W1[:, :CH], W1[:, CH:]
    cstv = np.zeros((P, CW), np.float32)
    cstv[:, _C_W1AT:_C_W1AT + KT * CHANS] = (
        W1a.T.reshape(KT, P, CHANS).transpose(1, 0, 2).reshape(P, KT * CHANS))
    cstv[:, _C_W1QT:_C_W1QT + KT * CHANS] = (
        W1q.T.reshape(KT, P, CHANS).transpose(1, 0, 2).reshape(P, KT * CHANS))
    for j in range(4):
        sl = slice(32 * j, 32 * j + CHANS)
        cstv[sl, _C_W2 + 32 * j:_C_W2 + 32 * j + CHANS] = W2.T
        cstv[sl, _C_W3 + 32 * j:_C_W3 + 32 * j + CHANS] = W3.T
        cstv[sl, _C_W4 + j] = W4[0]
        cstv[sl, _C_B2] = b2
        cstv[sl, _C_B3] = b3
        # selector: lhsT rows i, cols 32j+i -> replicates [18, n] into quads
        cstv[:CHANS, _C_REP + 32 * j:_C_REP + 32 * j + CHANS] = np.eye(
            CHANS, dtype=np.float32)
    cstv[:CHANS, _C_B1] = b1
    cstv[:4, _C_B4] = float(b4[0])

    # ---- per-core slices ----
    in_maps = []
    meta = []
    for b in range(B):
        aw = active[b]
        awp = np.zeros(NS, np.int64)
        if len(aw):
            awp[:len(aw)] = aw
            awp[len(aw):] = aw[0]
        # acquiring features for the active wi columns: [CH, 2*NS]
        qks = np.ascontiguousarray(
            acquiring_kspace[b][:, :, awp, :].reshape(CH, 2 * NS))
        for s in range(2):
            w0 = int(left[b]) + s * NWC
            w1e = max(min(w0 + NWC, W), w0)
            buf = np.zeros((C, H, NWC, 2), np.float32)
            cstc = cstv.copy()
            if w0 < W and w1e > w0:
                buf[:, :, :w1e - w0, :] = acquired_kspace[b, :, :, w0:w1e, :]
                d = denom[b] if denom[b] != 0 else 1.0
                cstc[:4, _C_CM:_C_CM + (w1e - w0)] = (
                    cmask[b, w0:w1e].astype(np.float32) / d)[None, :]
            aks = buf.reshape(CH, 2 * NWC)
            in_maps.append(dict(aks=aks, qks=qks, cst=cstc))
            meta.append((b, s))

    key = (NWC, NL)
    if key not in _prog_cache:
        _prog_cache[key] = _build_program(NWC, NL)
    nc = _prog_cache[key]

    trace = bool(int(os.environ.get("CABSK_TRACE", "0")))
    tmpdir = os.environ.get("CABSK_TMPDIR") or None
    if tmpdir:
        import tempfile
        tmpdir = tempfile.mkdtemp(dir=tmpdir)
    if os.environ.get("CABSK_SIM", "0") == "1":
        res = _run_sim(nc, in_maps)
    else:
        res = run_bass_kernel_spmd(nc, in_maps, core_ids=list(range(NCORES)),
                                   trace=trace, tmpdir=tmpdir)
    LAST_RESULTS = res

    heat = np.zeros((B, W), np.float32)
    for ci, (b, s) in enumerate(meta):
        hpv = res.results[ci]["hp"]          # [4, NL]
        aw = active[b]
        for t in range(len(aw)):
            heat[b, aw[t]] += hpv[t // NL, t % NL]
    out[:] = heat[:, None, :]
    return out



# revision 10
# speedup vs baseline: 1.0650x; 1.0650x over previous
"""Trainium2 Bass kernel for nn_Discriminator_80195629351349.

Pairwise-column MLP discriminator over k-space columns.

Math (matching the jax reference):
  F[b, w, ch] = |kspace[b, c, h, w]|  (ch = c*H + h)
  Pq = Fq @ W1[:, :CH].T ;  Pa = Fa @ W1[:, CH:].T          [B, W, 18]
  out[b, wi, wc] = sigmoid(W4 @ r3 + b4),  r3 = relu-chain of
                   relu(Pq[wi] + Pa[wc] + b1) through W2, W3
  heat[b, wi] = sum_wc out[b, wi, wc] * cmask[b, wc] / denom[b]
  result[b, h, w] = heat[b, w] if acquiring_mask[b, w] > 0 else 0

Only columns wi with acquiring_mask>0 (16 of 384) contribute to the
output, and the wc sum runs only over [left, right) (191 of 384
columns), so the kernel computes exactly that slice.

Sharding: 8 cores = (batch b in 0..3) x (wc half s in 0..1). Each core
gets its slice of acquired/acquiring k-space columns, pre-packed on
the host into bf16 in the exact SBUF layout (so every DMA is 128 fat
contiguous packets), computes features + all pair MLP evaluations
on-device, and returns partial heat sums [4, NL]. Host combines and
divides by denom.

On-device layout: the 18-channel MLP is packed 4x block-diagonal
across the 128 partitions (quadrant j = partitions 32j..32j+17), so
layers 2-4 are single matmuls with N = NL*NWC <= 512 free columns.
The q (acquiring) columns ride in the same feature stream as the a
(acquired) columns: per k-tile the rhs is [128, NWC+NS] and the W1
weights are concatenated [W1q_k | W1a_k] so one matmul chain yields
both Pq (rows 0:18, cols NWC:) and Pa (rows 18:36, cols :NWC).

Padding: wc slots beyond the valid span get a -30000 penalty added to
their pre-relu pair activations (one accumulate-matmul against a
constant indicator row), forcing h1=0 there, so their sigmoid output
is exactly sigmoid(b4); the host subtracts that known constant.
"""

import math
import os

import numpy as np
import ml_dtypes

BF16 = ml_dtypes.bfloat16

B, C, H, W = 4, 8, 384, 384
CH = C * H            # 3072 features per column
P = 128               # SBUF partitions
KT = CH // P          # 24 contraction tiles
CHANS = 18            # MLP width
W1W = 32 + CHANS      # W1cat lhsT width: Pq rows 0:18, Pa rows 32:50
NCORES = 8
NCHUNK = 2            # k-space stream chunks (DMA/compute overlap)
KPC = KT // NCHUNK    # k-tiles per chunk

_prog_cache: dict = {}
LAST_RESULTS = None   # BassKernelResults of the most recent run (for test.py)


def _cst_layout(NWC):
    """Column offsets of the constant block [128, CW] (bf16)."""
    o = {}
    off = 0
    o["W1"] = off; off += KT * W1W            # per-k [W1q_k | 0pad | W1a_k]
    o["W2BD"] = off; off += P                 # block-diag W2.T
    o["W3BD"] = off; off += P
    o["W4BD"] = off; off += 4
    o["REP"] = off; off += P                  # rows 0:18, eye at 32j offsets
    o["PEN"] = off; off += P                  # row 0 = -30000 everywhere
    o["IND"] = off; off += NWC                # row 0 = 1.0 at pad columns
    o["B1"] = off; off += 1
    o["B2"] = off; off += 1
    o["B3"] = off; off += 1
    o["B4"] = off; off += 1
    o["CW"] = off
    return o


def _build_program(NWC: int, NL: int):
    """Build the SPMD Bass/Tile program for one core.

    NWC: number of wc (acquired) columns this core handles.
    NL:  wi slots per partition-quadrant (total wi slots = 4*NL).
    """
    import concourse.bass as bass
    import concourse.tile as tile
    from concourse import bacc, mybir

    f32 = mybir.dt.float32
    bf16 = mybir.dt.bfloat16
    NS = 4 * NL          # wi slots
    NCOL = NWC + NS      # feature columns per k-tile (a block then q block)
    NF = NL * NWC        # free columns of the pair block
    o = _cst_layout(NWC)
    CW = o["CW"]
    CPF = KPC * NCOL     # feature columns per chunk
    assert NF <= 512

    nc = bacc.Bacc("TRN2", debug=False)

    # ---- DRAM I/O (per-core; host packs exactly these layouts) ----
    # xks partition row: [chunk][re/im][k-in-chunk][col] -> contiguous 1D
    xks = nc.dram_tensor("xks", [P, NCHUNK, 2, CPF], bf16, kind="ExternalInput")
    cst = nc.dram_tensor("cst", [P, CW], bf16, kind="ExternalInput")
    hp = nc.dram_tensor("hp", [4, NL], f32, kind="ExternalOutput")

    AF = mybir.ActivationFunctionType
    ALU = mybir.AluOpType

    with tile.TileContext(nc) as tc:
        with (
            tc.tile_pool(name="consts", bufs=1) as consts,
            tc.tile_pool(name="xdata", bufs=NCHUNK) as xdata,
            tc.tile_pool(name="sq", bufs=NCHUNK) as sqp,
            tc.tile_pool(name="feat", bufs=1) as featp,
            tc.tile_pool(name="mlp", bufs=1) as mlp,
            tc.tile_pool(name="ps1", bufs=1, space="PSUM") as ps1,
            tc.tile_pool(name="ps2", bufs=1, space="PSUM") as ps2p,
            tc.tile_pool(name="ps3", bufs=1, space="PSUM") as ps3p,
            tc.tile_pool(name="ps4", bufs=1, space="PSUM") as ps4p,
            tc.tile_pool(name="psa", bufs=1, space="PSUM") as psap,
        ):
            # ---- ACT sqrt-table prefetch before any data lands ----
            dum = mlp.tile([1, 1], f32, tag="dum")
            nc.vector.memset(dum, 1.0)
            dsq = mlp.tile([1, 1], f32, tag="dsq")
            nc.scalar.sqrt(dsq, dum)
            # zero pq4's dead partitions (32j+18..32j+31) before use
            pq4 = mlp.tile([P, NL], f32, tag="pq4")
            nc.vector.memset(pq4, 0.0)

            # ---- input DMAs: chunk0 then chunk1 on the sync ring,
            # constants on the scalar ring (all 128 fat packets) ----
            xc = []
            for ci in range(NCHUNK):
                t = xdata.tile([P, 2, CPF], bf16, tag=f"x{ci}")
                nc.sync.dma_start(out=t, in_=xks[:, ci])
                xc.append(t)
            cst_s = consts.tile([P, CW], bf16, tag="cst")
            nc.scalar.dma_start(out=cst_s, in_=cst[:])

            # biases as f32 (tensor_scalar/activation scalar operands
            # must be float32); B1..B4 are 4 consecutive cst columns
            bias_f = mlp.tile([P, 4], f32, tag="biasf")
            nc.vector.tensor_copy(bias_f, cst_s[:, o["B1"]:o["B1"] + 4])
            b1_s = bias_f[0:CHANS, 0:1]
            b2_s = bias_f[:, 1:2]
            b3_s = bias_f[:, 2:3]
            b4_s = bias_f[0:4, 3:4]

            # ---- features: |z| = sqrt(re^2 + im^2), then W1 matmuls ----
            feat = featp.tile([P, KT * NCOL], bf16, tag="feat")
            psw1 = ps1.tile([W1W, NCOL], f32, tag="psw1")
            for ci in range(NCHUNK):
                sq_re = sqp.tile([P, CPF], bf16, tag=f"sqre{ci}")
                nc.vector.tensor_mul(sq_re, xc[ci][:, 0], xc[ci][:, 0])
                sq_im = sqp.tile([P, CPF], bf16, tag=f"sqim{ci}")
                nc.vector.tensor_mul(sq_im, xc[ci][:, 1], xc[ci][:, 1])
                m2 = sqp.tile([P, CPF], bf16, tag=f"m2{ci}")
                nc.vector.tensor_add(m2, sq_re, sq_im)
                nc.scalar.sqrt(feat[:, ci * CPF:(ci + 1) * CPF], m2)
                for kk in range(KPC):
                    k = ci * KPC + kk
                    nc.tensor.matmul(
                        out=psw1,
                        lhsT=cst_s[:, o["W1"] + k * W1W:
                                   o["W1"] + (k + 1) * W1W],
                        rhs=feat[:, k * NCOL:(k + 1) * NCOL],
                        start=(k == 0),
                        stop=(k == KT - 1),
                    )

            # sigmoid ACT-table prefetch while the W1 tail runs
            dsg = mlp.tile([1, 1], f32, tag="dsg")
            nc.scalar.activation(out=dsg, in_=dum, func=AF.Sigmoid)

            # ---- split psw1 into Pq (+b1) and Pa; spread into quadrants ----
            pq_s = mlp.tile([CHANS, NS], f32, tag="pq")
            nc.vector.tensor_scalar(out=pq_s, in0=psw1[0:CHANS, NWC:NCOL],
                                    scalar1=b1_s, scalar2=None, op0=ALU.add)
            pa_s = mlp.tile([CHANS, NWC], bf16, tag="pa")
            nc.vector.tensor_copy(pa_s, psw1[32:32 + CHANS, 0:NWC])

            # pq4[32j+i, lw] = pq_s[i, j*NL+lw]: quadrant spread via 4
            # small DMAs on otherwise-idle queues
            for j, eng in zip(range(4), (nc.sync, nc.scalar, nc.gpsimd,
                                         nc.sync)):
                eng.dma_start(out=pq4[32 * j:32 * j + CHANS, :],
                              in_=pq_s[:, j * NL:(j + 1) * NL])

            # pa4 = REP.T @ pa (replicate into quadrants) + pad penalty
            pa4 = psap.tile([P, NWC], f32, tag="pa4")
            nc.tensor.matmul(out=pa4, lhsT=cst_s[0:CHANS, o["REP"]:o["REP"] + P],
                             rhs=pa_s, start=True, stop=False)
            nc.tensor.matmul(out=pa4, lhsT=cst_s[0:1, o["PEN"]:o["PEN"] + P],
                             rhs=cst_s[0:1, o["IND"]:o["IND"] + NWC],
                             start=False, stop=True)

            # ---- pair MLP ----
            h1 = mlp.tile([P, NF], bf16, tag="h1")
            for lw in range(NL):
                nc.vector.tensor_scalar(
                    out=h1[:, lw * NWC:(lw + 1) * NWC],
                    in0=pa4,
                    scalar1=pq4[:, lw:lw + 1],
                    scalar2=0.0,
                    op0=ALU.add,
                    op1=ALU.max,
                )
            psum2 = ps2p.tile([P, NF], f32, tag="ps2")
            nc.tensor.matmul(out=psum2, lhsT=cst_s[:, o["W2BD"]:o["W2BD"] + P],
                             rhs=h1, start=True, stop=True)
            h2 = mlp.tile([P, NF], bf16, tag="h2")
            nc.vector.tensor_scalar(out=h2, in0=psum2, scalar1=b2_s,
                                    scalar2=0.0, op0=ALU.add, op1=ALU.max)
            psum3 = ps3p.tile([P, NF], f32, tag="ps3")
            nc.tensor.matmul(out=psum3, lhsT=cst_s[:, o["W3BD"]:o["W3BD"] + P],
                             rhs=h2, start=True, stop=True)
            h3 = mlp.tile([P, NF], bf16, tag="h3")
            nc.vector.tensor_scalar(out=h3, in0=psum3, scalar1=b3_s,
                                    scalar2=0.0, op0=ALU.add, op1=ALU.max)
            psum4 = ps4p.tile([4, NF], f32, tag="ps4")
            nc.tensor.matmul(out=psum4, lhsT=cst_s[:, o["W4BD"]:o["W4BD"] + 4],
                             rhs=h3, start=True, stop=True)
            sig = mlp.tile([4, NF], bf16, tag="sig")
            nc.scalar.activation(out=sig, in_=psum4, func=AF.Sigmoid,
                                 bias=b4_s, scale=1.0)

            # ---- heat[j, lw] = sum_c sig[j, lw*NWC + c] ----
            hp_s = mlp.tile([4, NL], f32, tag="hps")
            nc.vector.reduce_sum(hp_s, sig.rearrange("p (l c) -> p l c", l=NL),
                                 axis=mybir.AxisListType.X)
            nc.sync.dma_start(out=hp[:], in_=hp_s)

    nc.finalize()
    return nc


def _run_sim(nc, in_maps):
    """CoreSim (CPU instruction simulator) path for local dev testing."""
    from concourse.bass_interp import MultiCoreSim
    from concourse.bass_utils import BassKernelResults

    sim = MultiCoreSim(nc, num_cores=len(in_maps))
    for core_id, core in sim.cores.items():
        for name, arr in in_maps[core_id].items():
            core.tensor(name)[:] = arr
    sim.simulate()
    results = [
        {"hp": np.array(sim.cores[i].tensor("hp"))} for i in range(len(in_maps))
    ]
    return BassKernelResults(results=results, instructions_and_trace=None,
                             profile_json=None, exec_time_ns=None)


def _mask_geometry(acquired_mask, acquiring_mask):
    """Replicates the reference's left/right/cmask/denom logic exactly."""
    am = np.asarray(acquired_mask, np.float32)
    qm = np.asarray(acquiring_mask, np.float32)
    mid = W // 2
    right = mid + np.argmax(am[:, mid:] < 1.0, axis=1)
    left = np.argmax(am[:, :mid][:, ::-1] < 1.0, axis=1) + 1
    cols = np.arange(W)
    cmask = (cols[None, :] >= left[:, None]) & (cols[None, :] < right[:, None])
    denom = (right - left).astype(np.float32)
    active = [np.nonzero(qm[b] > 0)[0] for b in range(B)]
    return left.astype(int), right.astype(int), cmask, denom, active


def kernel(acquired_kspace, acquiring_kspace, acquired_mask, acquiring_mask,
           W1, b1, W2, b2, W3, b3, W4, b4):
    global LAST_RESULTS
    from concourse.bass_utils import run_bass_kernel_spmd

    acquired_kspace = np.asarray(acquired_kspace, np.float32)
    acquiring_kspace = np.asarray(acquiring_kspace, np.float32)
    W1 = np.asarray(W1, np.float32)
    b1 = np.asarray(b1, np.float32)
    W2 = np.asarray(W2, np.float32)
    b2 = np.asarray(b2, np.float32)
    W3 = np.asarray(W3, np.float32)
    b3 = np.asarray(b3, np.float32)
    W4 = np.asarray(W4, np.float32)
    b4 = np.asarray(b4, np.float32)

    left, right, cmask, denom, active = _mask_geometry(acquired_mask, acquiring_mask)

    nmax = max(len(a) for a in active)
    out = np.zeros((B, H, W), np.float32)
    if nmax == 0:
        return out

    span = max(int((right - left).max()), 1)
    NL = max(1, math.ceil(nmax / 4))          # wi slots per quadrant
    NWC = max(1, math.ceil(span / 2))         # wc columns per core
    NWC = ((NWC + KPC - 1) // KPC) * KPC if False else NWC
    NS = 4 * NL
    NCOL = NWC + NS
    assert NL * NWC <= 512, (NL, NWC)

    o = _cst_layout(NWC)

    # ---- shared constant block [128, CW] bf16 ----
    W1q, W1a = W1[:, :CH], W1[:, CH:]
    cstv = np.zeros((P, o["CW"]), np.float32)
    # W1cat per k-tile: lhsT[p, k*36 + j] = W1q[j, k*128+p] (j<18) else W1a
    w1q_t = W1q.T.reshape(KT, P, CHANS)       # [k, p, j]
    w1a_t = W1a.T.reshape(KT, P, CHANS)
    w1cat = np.zeros((KT, P, W1W), np.float32)
    w1cat[:, :, :CHANS] = w1q_t
    w1cat[:, :, 32:32 + CHANS] = w1a_t
    cstv[:, o["W1"]:o["W1"] + KT * W1W] = (
        w1cat.transpose(1, 0, 2).reshape(P, KT * W1W))
    for j in range(4):
        sl = slice(32 * j, 32 * j + CHANS)
        cstv[sl, o["W2BD"] + 32 * j:o["W2BD"] + 32 * j + CHANS] = W2.T
        cstv[sl, o["W3BD"] + 32 * j:o["W3BD"] + 32 * j + CHANS] = W3.T
        cstv[sl, o["W4BD"] + j] = W4[0]
        cstv[sl, o["B2"]] = b2
        cstv[sl, o["B3"]] = b3
        cstv[:CHANS, o["REP"] + 32 * j:o["REP"] + 32 * j + CHANS] = np.eye(
            CHANS, dtype=np.float32)
    cstv[0, o["PEN"]:o["PEN"] + P] = -30000.0
    cstv[:CHANS, o["B1"]] = b1
    cstv[:4, o["B4"]] = float(b4[0])

    # ---- per-core packed inputs ----
    in_maps = []
    meta = []
    npad = np.zeros((B, 2), np.int64)
    for b in range(B):
        aw = active[b]
        awp = np.zeros(NS, np.int64)
        if len(aw):
            awp[:len(aw)] = aw
            awp[len(aw):] = aw[0]
        # acquiring features for active wi columns: [KT, P, NS, 2]
        Q = acquiring_kspace[b].reshape(CH, W, 2)[:, awp, :].reshape(
            KT, P, NS, 2)
        for s in range(2):
            w0 = int(left[b]) + s * NWC
            w1e = max(min(w0 + NWC, int(right[b])), w0)
            nv = w1e - w0
            npad[b, s] = NWC - nv
            A = np.zeros((KT, P, NWC, 2), np.float32)
            if nv > 0:
                A[:, :, :nv, :] = acquired_kspace[b].reshape(CH, W, 2)[
                    :, w0:w1e, :].reshape(KT, P, nv, 2)
            F = np.concatenate([A, Q], axis=2)      # [KT, P, NCOL, 2]
            # -> [P, chunk, r, k-in-chunk, col]
            X = F.reshape(NCHUNK, KPC, P, NCOL, 2).transpose(2, 0, 4, 1, 3)
            xarr = np.ascontiguousarray(X).astype(BF16).reshape(
                P, NCHUNK, 2, KPC * NCOL)
            cstc = cstv.copy()
            if nv < NWC:
                cstc[0, o["IND"] + nv:o["IND"] + NWC] = 1.0
            in_maps.append(dict(xks=xarr, cst=cstc.astype(BF16)))
            meta.append((b, s))

    key = (NWC, NL)
    if key not in _prog_cache:
        _prog_cache[key] = _build_program(NWC, NL)
    nc = _prog_cache[key]

    trace = bool(int(os.environ.get("CABSK_TRACE", "0")))
    tmpdir = os.environ.get("CABSK_TMPDIR") or None
    if tmpdir:
        import tempfile
        tmpdir = tempfile.mkdtemp(dir=tmpdir)
    if os.environ.get("CABSK_SIM", "0") == "1":
        res = _run_sim(nc, in_maps)
    else:
        res = run_bass_kernel_spmd(nc, in_maps, core_ids=list(range(NCORES)),
                                   trace=trace, tmpdir=tmpdir)
    LAST_RESULTS = res

    sig_b4 = 1.0 / (1.0 + math.exp(-float(b4[0])))  # pad columns' output
    heat = np.zeros((B, W), np.float32)
    for ci, (b, s) in enumerate(meta):
        hpv = np.asarray(res.results[ci]["hp"], np.float32)   # [4, NL]
        aw = active[b]
        corr = float(npad[b, s]) * sig_b4
        for t in range(len(aw)):
            heat[b, aw[t]] += hpv[t // NL, t % NL] - corr
    heat /= np.maximum(denom, 1.0)[:, None]
    out[:] = heat[:, None, :]
    return out


# revision 16
# speedup vs baseline: 1.3540x; 1.2715x over previous
"""Trainium2 Bass kernel for nn_Discriminator_80195629351349.

Pairwise-column MLP discriminator over k-space columns.

Math (matching the jax reference):
  F[b, w, ch] = |kspace[b, c, h, w]|  (ch = c*H + h)
  Pq = Fq @ W1[:, :CH].T ;  Pa = Fa @ W1[:, CH:].T          [B, W, 18]
  out[b, wi, wc] = sigmoid(W4 @ r3 + b4),  r3 = relu-chain of
                   relu(Pq[wi] + Pa[wc] + b1) through W2, W3
  heat[b, wi] = sum_wc out[b, wi, wc] * cmask[b, wc] / denom[b]
  result[b, h, w] = heat[b, w] if acquiring_mask[b, w] > 0 else 0

Only columns wi with acquiring_mask>0 (16 of 384) contribute to the
output, and the wc sum runs only over [left, right) (191 of 384
columns), so the kernel computes exactly that slice.

Sharding: 8 cores = (batch b in 0..3) x (wc half s in 0..1). Each core
gets its slice of acquired/acquiring k-space columns, pre-packed on
the host into bf16 in the exact SBUF layout (so every DMA is 128 fat
contiguous packets), computes features + all pair MLP evaluations
on-device, and returns partial heat sums [4, NL]. Host combines and
divides by denom.

On-device layout: the 18-channel MLP is packed 4x block-diagonal
across the 128 partitions (quadrant j = partitions 32j..32j+17), so
layers 2-4 are single matmuls with N = NL*NWC <= 512 free columns.
The q (acquiring) columns ride in the same feature stream as the a
(acquired) columns: per k-tile the rhs is [128, NWC+NS] and the W1
lhsT is [W1q_k | zeros | W1a_k] (50 wide) so one matmul chain yields
Pq (rows 0:18, cols NWC:) and Pa (rows 32:50, cols :NWC) in one PSUM.

The pair pre-activation H[32j+i, lw*NWC+c] = Pa[i,c] + Pq[i,j*NL+lw]
(+b1, and -30000 on pad columns) is built entirely on the PE: Pq is
transposed via the PE transpose path, scattered into a per-slot lhsT
L (tiny same-partition copies), and one matmul against a constant
selector adds the right Pq value to every pair column; four more
column-slice matmuls against REP replicate Pa. No SBUF->SBUF DMAs.

Padding: pad columns get -30000 via the selector's extra all-ones row,
forcing h1=0 there, so their sigmoid output is exactly sigmoid(b4);
the host subtracts that known constant.
"""

import math
import os

import numpy as np
import ml_dtypes

BF16 = ml_dtypes.bfloat16

B, C, H, W = 4, 8, 384, 384
CH = C * H            # 3072 features per column
P = 128               # SBUF partitions
KT = CH // P          # 24 contraction tiles
CHANS = 18            # MLP width
W1W = 32 + CHANS      # W1cat lhsT width: Pq rows 0:18, Pa rows 32:50
NCORES = 8
NCHUNK = 3            # k-space stream chunks (DMA/compute overlap)
KPC = KT // NCHUNK    # k-tiles per chunk
GPIM = 384            # im^2 columns per chunk offloaded to GpSimd

_prog_cache: dict = {}
LAST_RESULTS = None   # BassKernelResults of the most recent run (for test.py)


def _cst_layout(NWC, NS, NF):
    """Column offsets of the constant block [128, CW] (bf16)."""
    o = {}
    off = 0
    o["W1"] = off; off += KT * W1W            # per-k [W1q_k | 0pad | W1a_k]
    o["W2BD"] = off; off += P                 # block-diag W2.T
    o["W3BD"] = off; off += P
    o["W4BD"] = off; off += 4
    o["REP"] = off; off += P                  # rows 0:18, eye at 32j offsets
    o["SEL"] = off; off += NF                 # row 32j+lw: slot selector
    o["PADP"] = off; off += NWC               # rows 0:18: -30000 at pad cols
    o["B1"] = off; off += 1
    o["B2"] = off; off += 1
    o["B3"] = off; off += 1
    o["B4"] = off; off += 1
    o["CW"] = off
    return o


def _build_program(NWC: int, NL: int):
    """Build the SPMD Bass/Tile program for one core.

    NWC: number of wc (acquired) columns this core handles.
    NL:  wi slots per partition-quadrant (total wi slots = 4*NL).
    """
    import concourse.bass as bass
    import concourse.tile as tile
    from concourse import bacc, mybir

    f32 = mybir.dt.float32
    bf16 = mybir.dt.bfloat16
    NS = 4 * NL          # wi slots
    NCOL = NWC + NS      # feature columns per k-tile (a block then q block)
    NF = NL * NWC        # free columns of the pair block
    o = _cst_layout(NWC, NS, NF)
    CW = o["CW"]
    CPF = KPC * NCOL     # feature columns per chunk
    assert NF <= 512

    nc = bacc.Bacc("TRN2", debug=False)

    # ---- DRAM I/O (per-core; host packs exactly these layouts) ----
    # xks partition row: [chunk][re/im][k-in-chunk][col] -> contiguous 1D
    xks = nc.dram_tensor("xks", [P, NCHUNK, 2, CPF], bf16, kind="ExternalInput")
    cst = nc.dram_tensor("cst", [P, CW], bf16, kind="ExternalInput")
    hp = nc.dram_tensor("hp", [4, NL], f32, kind="ExternalOutput")

    AF = mybir.ActivationFunctionType
    ALU = mybir.AluOpType

    halves = (NL % 2 == 0 and NL >= 2)
    with tile.TileContext(nc) as tc:
        with (
            tc.tile_pool(name="consts", bufs=1) as consts,
            tc.tile_pool(name="xdata", bufs=NCHUNK) as xdata,
            tc.tile_pool(name="sq", bufs=2) as sqp,
            tc.tile_pool(name="feat", bufs=1) as featp,
            tc.tile_pool(name="mlp", bufs=1) as mlp,
            tc.tile_pool(name="ps1", bufs=1, space="PSUM") as ps1,
            tc.tile_pool(name="psT", bufs=1, space="PSUM") as psT,
            tc.tile_pool(name="psH", bufs=1, space="PSUM") as psH,
            tc.tile_pool(name="ps2", bufs=1, space="PSUM") as ps2p,
            tc.tile_pool(name="ps3", bufs=1, space="PSUM") as ps3p,
            tc.tile_pool(name="ps4", bufs=1, space="PSUM") as ps4p,
        ):
            # ---- ACT sqrt-table prefetch before any data lands ----
            dum = mlp.tile([1, 1], f32, tag="dum")
            nc.vector.memset(dum, 1.0)
            dsq = mlp.tile([1, 1], f32, tag="dsq")
            nc.scalar.sqrt(dsq, dum)
            # L: per-slot lhsT for the Pq scatter matmul; row 32j+lw
            # gets pqT quadrant block j (32-aligned partition bases)
            L = mlp.tile([P, P], bf16, tag="L")
            nc.vector.memset(L, 0.0)

            # ---- input DMAs: chunks on the sync ring, constants on the
            # scalar ring (every packet is a fat contiguous row) ----
            xc = []
            for ci in range(NCHUNK):
                t = xdata.tile([P, 2, CPF], bf16, tag=f"x{ci}")
                nc.sync.dma_start(out=t, in_=xks[:, ci])
                xc.append(t)
            cst_s = consts.tile([P, CW], bf16, tag="cst")
            nc.scalar.dma_start(out=cst_s, in_=cst[:])

            # ---- features: |z| = sqrt(re^2 + im^2), then W1 matmuls ----
            feat = featp.tile([P, KT * NCOL], bf16, tag="feat")
            psw1 = ps1.tile([W1W, NCOL], f32, tag="psw1")
            gpi = min(GPIM, CPF)
            for ci in range(NCHUNK):
                sq_re = sqp.tile([P, CPF], bf16, tag=f"sqre{ci}")
                nc.vector.tensor_mul(sq_re, xc[ci][:, 0], xc[ci][:, 0])
                sq_im = sqp.tile([P, CPF], bf16, tag=f"sqim{ci}")
                # GpSimd takes a slice of the im^2 work (it runs at ~0.4x
                # DVE speed; this keeps DVE at the DMA cadence)
                nc.gpsimd.tensor_mul(sq_im[:, :gpi],
                                     xc[ci][:, 1, :gpi], xc[ci][:, 1, :gpi])
                if gpi < CPF:
                    nc.vector.tensor_mul(sq_im[:, gpi:],
                                         xc[ci][:, 1, gpi:], xc[ci][:, 1, gpi:])
                m2 = sqp.tile([P, CPF], bf16, tag=f"m2{ci}")
                nc.vector.tensor_add(m2, sq_re, sq_im)
                nc.scalar.sqrt(feat[:, ci * CPF:(ci + 1) * CPF], m2)
                for kk in range(KPC):
                    k = ci * KPC + kk
                    nc.tensor.matmul(
                        out=psw1,
                        lhsT=cst_s[:, o["W1"] + k * W1W:
                                   o["W1"] + (k + 1) * W1W],
                        rhs=feat[:, k * NCOL:(k + 1) * NCOL],
                        start=(k == 0),
                        stop=(k == KT - 1),
                    )

            # sigmoid ACT-table prefetch: chained on the last sqrt's output
            # so it runs after all sqrts (exactly one table swap)
            dsg = mlp.tile([1, 1], f32, tag="dsg")
            nc.scalar.activation(out=dsg, in_=feat[0:1, KT * NCOL - 1:KT * NCOL],
                                 func=AF.Sigmoid)

            # biases as f32 (scalar operands must be float32); emitted late
            # so this wait doesn't block the DVE queue
            bias_f = mlp.tile([P, 4], f32, tag="biasf")
            nc.gpsimd.tensor_copy(bias_f, cst_s[:, o["B1"]:o["B1"] + 4])
            b1_s = bias_f[0:CHANS, 0:1]
            b2_s = bias_f[:, 1:2]
            b3_s = bias_f[:, 2:3]
            b4_s = bias_f[0:4, 3:4]

            # ---- split psw1 into Pq (+b1) and Pa ----
            pq_s = mlp.tile([CHANS, NS], bf16, tag="pq")
            nc.vector.tensor_scalar(out=pq_s, in0=psw1[0:CHANS, NWC:NCOL],
                                    scalar1=b1_s, scalar2=None, op0=ALU.add)
            # Pa extraction; the add also injects -30000 into pad columns
            pa_s = mlp.tile([CHANS, NWC], bf16, tag="pa")
            nc.vector.tensor_add(pa_s, psw1[32:32 + CHANS, 0:NWC],
                                 cst_s[0:CHANS, o["PADP"]:o["PADP"] + NWC])

            # ---- Pq quadrant scatter without DMA: 4 per-quadrant PE
            # transposes into one PSUM bank (base 0), then 32-aligned
            # copies into L ----
            pqT = psT.tile([NL, 4 * CHANS], bf16, tag="pqT")
            eye = cst_s[0:CHANS, o["REP"]:o["REP"] + CHANS]
            for j in range(4):
                nc.tensor.matmul(
                    out=pqT[:, j * CHANS:(j + 1) * CHANS],
                    lhsT=pq_s[:, j * NL:(j + 1) * NL], rhs=eye,
                    is_transpose=True, start=(j == 0), stop=(j == 3),
                    skip_group_check=True)
            for j in range(4):
                nc.vector.tensor_copy(
                    L[32 * j:32 * j + NL, 32 * j:32 * j + CHANS],
                    pqT[:, j * CHANS:(j + 1) * CHANS])

            # ---- H = Pa (replicated) + Pq (scattered) - 30000*pad ----
            Hps = psH.tile([P, NF], f32, tag="H")
            for lw in range(NL):
                nc.tensor.matmul(out=Hps[:, lw * NWC:(lw + 1) * NWC],
                                 lhsT=cst_s[0:CHANS, o["REP"]:o["REP"] + P],
                                 rhs=pa_s, start=(lw == 0), stop=False,
                                 skip_group_check=True)
            nc.tensor.matmul(out=Hps,
                             lhsT=L,
                             rhs=cst_s[:, o["SEL"]:o["SEL"] + NF],
                             start=False, stop=True, skip_group_check=True)

            # ---- pair MLP (two column-half pipelines when NL is even) ----
            h1 = mlp.tile([P, NF], bf16, tag="h1")
            h2 = mlp.tile([P, NF], bf16, tag="h2")
            h3 = mlp.tile([P, NF], bf16, tag="h3")
            sig = mlp.tile([4, NF], bf16, tag="sig")
            psum2 = ps2p.tile([P, NF], f32, tag="ps2")
            psum3 = ps3p.tile([P, NF], f32, tag="ps3")
            psum4 = ps4p.tile([4, NF], f32, tag="ps4")
            hp_s = mlp.tile([4, NL], f32, tag="hps")
            HNF = NF // 2 if halves else NF
            HNL = NL // 2 if halves else NL
            for hf in range(2 if halves else 1):
                sl = slice(hf * HNF, hf * HNF + HNF)
                nc.vector.tensor_scalar(out=h1[:, sl], in0=Hps[:, sl],
                                        scalar1=0.0, scalar2=None, op0=ALU.max)
                nc.tensor.matmul(out=psum2[:, sl],
                                 lhsT=cst_s[:, o["W2BD"]:o["W2BD"] + P],
                                 rhs=h1[:, sl], start=(hf == 0), stop=True,
                                 skip_group_check=True)
                nc.vector.tensor_scalar(out=h2[:, sl], in0=psum2[:, sl],
                                        scalar1=b2_s, scalar2=0.0,
                                        op0=ALU.add, op1=ALU.max)
                nc.tensor.matmul(out=psum3[:, sl],
                                 lhsT=cst_s[:, o["W3BD"]:o["W3BD"] + P],
                                 rhs=h2[:, sl], start=(hf == 0), stop=True,
                                 skip_group_check=True)
                nc.vector.tensor_scalar(out=h3[:, sl], in0=psum3[:, sl],
                                        scalar1=b3_s, scalar2=0.0,
                                        op0=ALU.add, op1=ALU.max)
                nc.tensor.matmul(out=psum4[:, sl],
                                 lhsT=cst_s[:, o["W4BD"]:o["W4BD"] + 4],
                                 rhs=h3[:, sl], start=(hf == 0), stop=True,
                                 skip_group_check=True)
                nc.scalar.activation(out=sig[:, sl], in_=psum4[:, sl],
                                     func=AF.Sigmoid, bias=b4_s, scale=1.0)
                # heat[j, lw] = sum_c sig[j, lw*NWC + c]
                nc.vector.reduce_sum(
                    hp_s[:, hf * HNL:hf * HNL + HNL],
                    sig[:, sl].rearrange("p (l c) -> p l c", l=HNL),
                    axis=mybir.AxisListType.X)
            nc.sync.dma_start(out=hp[:], in_=hp_s)

    nc.finalize()
    return nc


def _run_sim(nc, in_maps):
    """CoreSim (CPU instruction simulator) path for local dev testing."""
    from concourse.bass_interp import MultiCoreSim
    from concourse.bass_utils import BassKernelResults

    sim = MultiCoreSim(nc, num_cores=len(in_maps))
    for core_id, core in sim.cores.items():
        for name, arr in in_maps[core_id].items():
            core.tensor(name)[:] = arr
    sim.simulate()
    results = [
        {"hp": np.array(sim.cores[i].tensor("hp"))} for i in range(len(in_maps))
    ]
    return BassKernelResults(results=results, instructions_and_trace=None,
                             profile_json=None, exec_time_ns=None)


def _mask_geometry(acquired_mask, acquiring_mask):
    """Replicates the reference's left/right/cmask/denom logic exactly."""
    am = np.asarray(acquired_mask, np.float32)
    qm = np.asarray(acquiring_mask, np.float32)
    mid = W // 2
    right = mid + np.argmax(am[:, mid:] < 1.0, axis=1)
    left = np.argmax(am[:, :mid][:, ::-1] < 1.0, axis=1) + 1
    cols = np.arange(W)
    cmask = (cols[None, :] >= left[:, None]) & (cols[None, :] < right[:, None])
    denom = (right - left).astype(np.float32)
    active = [np.nonzero(qm[b] > 0)[0] for b in range(B)]
    return left.astype(int), right.astype(int), cmask, denom, active


def kernel(acquired_kspace, acquiring_kspace, acquired_mask, acquiring_mask,
           W1, b1, W2, b2, W3, b3, W4, b4):
    global LAST_RESULTS
    from concourse.bass_utils import run_bass_kernel_spmd

    acquired_kspace = np.asarray(acquired_kspace, np.float32)
    acquiring_kspace = np.asarray(acquiring_kspace, np.float32)
    W1 = np.asarray(W1, np.float32)
    b1 = np.asarray(b1, np.float32)
    W2 = np.asarray(W2, np.float32)
    b2 = np.asarray(b2, np.float32)
    W3 = np.asarray(W3, np.float32)
    b3 = np.asarray(b3, np.float32)
    W4 = np.asarray(W4, np.float32)
    b4 = np.asarray(b4, np.float32)

    left, right, cmask, denom, active = _mask_geometry(acquired_mask, acquiring_mask)

    nmax = max(len(a) for a in active)
    out = np.zeros((B, H, W), np.float32)
    if nmax == 0:
        return out

    span = max(int((right - left).max()), 1)
    NL = max(1, math.ceil(nmax / 4))          # wi slots per quadrant
    NWC = max(1, math.ceil(span / 2))         # wc columns per core
    NS = 4 * NL
    NF = NL * NWC
    NCOL = NWC + NS
    assert NF <= 512, (NL, NWC)

    o = _cst_layout(NWC, NS, NF)

    # ---- shared constant block [128, CW] bf16 ----
    W1q, W1a = W1[:, :CH], W1[:, CH:]
    cstv = np.zeros((P, o["CW"]), np.float32)
    # W1cat per k-tile: lhsT[p, k*50 + j] = W1q[j, ...] (j<18),
    # W1a[j-32, ...] (j>=32)
    w1q_t = W1q.T.reshape(KT, P, CHANS)       # [k, p, j]
    w1a_t = W1a.T.reshape(KT, P, CHANS)
    w1cat = np.zeros((KT, P, W1W), np.float32)
    w1cat[:, :, :CHANS] = w1q_t
    w1cat[:, :, 32:32 + CHANS] = w1a_t
    cstv[:, o["W1"]:o["W1"] + KT * W1W] = (
        w1cat.transpose(1, 0, 2).reshape(P, KT * W1W))
    for j in range(4):
        sl = slice(32 * j, 32 * j + CHANS)
        cstv[sl, o["W2BD"] + 32 * j:o["W2BD"] + 32 * j + CHANS] = W2.T
        cstv[sl, o["W3BD"] + 32 * j:o["W3BD"] + 32 * j + CHANS] = W3.T
        cstv[sl, o["W4BD"] + j] = W4[0]
        cstv[sl, o["B2"]] = b2
        cstv[sl, o["B3"]] = b3
        cstv[:CHANS, o["REP"] + 32 * j:o["REP"] + 32 * j + CHANS] = np.eye(
            CHANS, dtype=np.float32)
    # slot selector: L row 32j+lw carries Pq of wi slot j*NL+lw; its
    # selector row lights up pair-column block lw
    for j in range(4):
        for lw in range(NL):
            cstv[32 * j + lw,
                 o["SEL"] + lw * NWC:o["SEL"] + (lw + 1) * NWC] = 1.0
    cstv[:CHANS, o["B1"]] = b1
    cstv[:4, o["B4"]] = float(b4[0])

    # ---- per-core packed inputs ----
    in_maps = []
    meta = []
    npad = np.zeros((B, 2), np.int64)
    for b in range(B):
        aw = active[b]
        awp = np.zeros(NS, np.int64)
        if len(aw):
            awp[:len(aw)] = aw
            awp[len(aw):] = aw[0]
        # acquiring features for active wi columns: [KT, P, NS, 2]
        Q = acquiring_kspace[b].reshape(CH, W, 2)[:, awp, :].reshape(
            KT, P, NS, 2)
        for s in range(2):
            w0 = int(left[b]) + s * NWC
            w1e = max(min(w0 + NWC, int(right[b])), w0)
            nv = w1e - w0
            npad[b, s] = NWC - nv
            A = np.zeros((KT, P, NWC, 2), np.float32)
            if nv > 0:
                A[:, :, :nv, :] = acquired_kspace[b].reshape(CH, W, 2)[
                    :, w0:w1e, :].reshape(KT, P, nv, 2)
            F = np.concatenate([A, Q], axis=2)      # [KT, P, NCOL, 2]
            # -> [P, chunk, r, k-in-chunk, col]
            X = F.reshape(NCHUNK, KPC, P, NCOL, 2).transpose(2, 0, 4, 1, 3)
            xarr = np.ascontiguousarray(X).astype(BF16).reshape(
                P, NCHUNK, 2, KPC * NCOL)
            cstc = cstv.copy()
            if nv < NWC:
                cstc[0:CHANS, o["PADP"] + nv:o["PADP"] + NWC] = -30000.0
            in_maps.append(dict(xks=xarr, cst=cstc.astype(BF16)))
            meta.append((b, s))

    key = (NWC, NL)
    if key not in _prog_cache:
        _prog_cache[key] = _build_program(NWC, NL)
    nc = _prog_cache[key]

    trace = bool(int(os.environ.get("CABSK_TRACE", "0")))
    tmpdir = os.environ.get("CABSK_TMPDIR") or None
    if tmpdir:
        import tempfile
        tmpdir = tempfile.mkdtemp(dir=tmpdir)
    if os.environ.get("CABSK_SIM", "0") == "1":
        res = _run_sim(nc, in_maps)
    else:
        res = run_bass_kernel_spmd(nc, in_maps, core_ids=list(range(NCORES)),
                                   trace=trace, tmpdir=tmpdir)
    LAST_RESULTS = res

    sig_b4 = 1.0 / (1.0 + math.exp(-float(b4[0])))  # pad columns' output
    heat = np.zeros((B, W), np.float32)
    for ci, (b, s) in enumerate(meta):
        hpv = np.asarray(res.results[ci]["hp"], np.float32)   # [4, NL]
        aw = active[b]
        corr = float(npad[b, s]) * sig_b4
        for t in range(len(aw)):
            heat[b, aw[t]] += hpv[t // NL, t % NL] - corr
    heat /= np.maximum(denom, 1.0)[:, None]
    out[:] = heat[:, None, :]
    return out
